# revision 53
# baseline (speedup 1.0000x reference)
"""Trainium2 Bass kernel for nn_AdaptiveChebBlock (8 NeuronCores).

Sharding: batch b = core//2 (4 batches), row-half j = core%2 (2048 rows each).
Each core computes its 2048 rows of the dynamic top-k adjacency + Chebyshev
propagation for its batch; pair collectives (AllGather over [2c,2c+1]) exchange
the degree vector and T1.
"""
import os, sys
os.environ.setdefault("JAX_PLATFORMS", "")
for _p in ("/root/.axon_site/_ro/trn_rl_repo", "/opt/trn_rl_repo"):
    if os.path.isdir(_p):
        if _p not in sys.path:
            sys.path.insert(0, _p)
        break  # use exactly one copy — mixing versions breaks imports

import numpy as np

import concourse.bass as bass
import concourse.bacc as bacc
import concourse.tile as tile
import concourse.mybir as mybir
import concourse.masks as masks
from concourse.bass_utils import run_bass_kernel_spmd

F32 = mybir.dt.float32
F16 = mybir.dt.float16
Alu = mybir.AluOpType
Act = mybir.ActivationFunctionType
AxX = mybir.AxisListType.X

KCHEB = 3
TOPK = 32
TELEPORT = 0.1
LN_EPS = 1e-5

# problem shape (hardcoded per spec)
BSZ, NFULL, DDIM = 4, 4096, 128
HDIM, ODIM = 128, 128
N_CORES = 8


class Cfg:
    def __init__(self, n_nodes, n_rows, use_cc, scalars, flags, gelu=True):
        self.n = n_nodes            # nodes this core sees (columns of A)
        self.r = n_rows             # rows this core owns
        self.NT = n_nodes // 128    # node tiles
        self.RT = n_rows // 128     # row tiles
        self.use_cc = use_cc        # emit pair collectives (8-core mode)
        self.c1, self.c2, self.tg = scalars
        # flags: which optional affine params are non-trivial
        self.lng, self.lnb, self.b1, self.b2 = flags
        self.gelu = gelu            # False only for CoreSim (no Gelu in interp)
        self.seg = 256              # top-k candidate segment width (0 = full-row rounds)
        self.mask_dve_mod = 1       # every k-th tile's mask on DVE, rest on gpsimd (1=all DVE)


def _emit(nc, tc, cfg):
    """Emit the whole per-core graph inside TileContext tc."""
    n, r, NT, RT = cfg.n, cfg.r, cfg.NT, cfg.RT
    c1, c2, tg = cfg.c1, cfg.c2, cfg.tg
    gelu_f = Act.Gelu if cfg.gelu else Act.Identity

    # ---- DRAM I/O -------------------------------------------------------
    xf = nc.dram_tensor("xf", [n, DDIM], F32, kind="ExternalInput")       # full batch slice
    xm = nc.dram_tensor("xm", [r, DDIM], F32, kind="ExternalInput")       # my rows
    w1e = nc.dram_tensor("w1e", [DDIM, HDIM], F32, kind="ExternalInput")
    w2e = nc.dram_tensor("w2e", [KCHEB * HDIM, ODIM], F32, kind="ExternalInput")
    lng_e = nc.dram_tensor("lng", [DDIM], F32, kind="ExternalInput")
    lnb_e = nc.dram_tensor("lnb", [DDIM], F32, kind="ExternalInput")
    b1_e = nc.dram_tensor("b1e", [HDIM], F32, kind="ExternalInput")
    b2_e = nc.dram_tensor("b2e", [ODIM], F32, kind="ExternalInput")
    out_e = nc.dram_tensor("out", [r, ODIM], F32, kind="ExternalOutput")

    # DRAM scratch
    h16_dram = nc.dram_tensor("h16_scr", [128, n], F16)        # h full nodes (g p), fp16
    dm_in = nc.dram_tensor("dm_in", [r], F32)
    t1_in = nc.dram_tensor("t1_in", [r, HDIM], F16)
    # NOTE: Shared addr_space is rejected for 2-rank groups; plain DRAM works.
    dm_out = nc.dram_tensor("dm_out", [n], F32)
    t1_out = nc.dram_tensor("t1_out", [n, HDIM], F16)
    groups = [[0, 1], [2, 3], [4, 5], [6, 7]]

    import contextlib
    stack = contextlib.ExitStack()
    const = stack.enter_context(tc.tile_pool(name="const", bufs=1))
    persist = stack.enter_context(tc.tile_pool(name="persist", bufs=1))

    id32 = const.tile([128, 128], F32, tag="id32")
    masks.make_identity(nc, id32[:])
    w1s = const.tile([DDIM, HDIM], F32, tag="w1s")
    nc.sync.dma_start(w1s[:], w1e[:])
    if cfg.lng:
        LNG = const.tile([128, DDIM], F32, tag="LNG")
        nc.sync.dma_start(LNG[:], lng_e.ap().partition_broadcast(128))
    if cfg.lnb:
        LNB = const.tile([128, DDIM], F32, tag="LNB")
        nc.sync.dma_start(LNB[:], lnb_e.ap().partition_broadcast(128))
    if cfg.b1:
        B1R = const.tile([128, HDIM], F32, tag="B1R")
        nc.sync.dma_start(B1R[:], b1_e.ap().partition_broadcast(128))

    # masked A transposed: MT[p, t, g, rr] = M16_tile_t[rr, g*128+p]
    # (row-tile-major so each tile's DMA-transpose destination is contiguous)
    MT = persist.tile([128, RT, NT, 128], F16, tag="MT")
    w2s = persist.tile([128, KCHEB, ODIM], F16, tag="w2s")
    degM = persist.tile([128, RT], F32, tag="degM")
    dmv = persist.tile([128, 6, RT], F32, tag="dmv")      # [deg|dm12|cdm1|q1|q2|cdm2]
    dm12oth = persist.tile([128, RT], F32, tag="dm12oth")  # partner-half dm12
    # own-rows h lives in persist so its DRAM load can issue right after the
    # feature pass, long before the pass-2 pools open (it gates Gown -> T1)
    hm16 = persist.tile([128, RT, HDIM], F16, tag="hm16")

    # =====================================================================
    # Batched feature pass: x tiles -> LN -> h=gelu(.@w1) -> xn -> xnT_dst
    # (3 sweeps, everything resident; used for the full batch and for the
    #  core's own rows)
    # =====================================================================
    early_stack = contextlib.ExitStack()
    early = early_stack.enter_context(tc.tile_pool(name="early", bufs=1))
    xnT = early.tile([128, n], F16, tag="xnT")

    def _moments(pool, nt, src_tiles, tagp, eps):
        """Per-tile mean + 1/sqrt(var+eps) via bn_stats (one DVE op per tile),
        plus the raw sum-of-squares. Returns (mu, rstd, ssq)."""
        bnst = pool.tile([128, nt, 6], F32, tag=tagp + "_bnst")
        for g in range(nt):
            nc.vector.bn_stats(bnst[:, g, :], src_tiles[:, g, :])
        me, mo = bnst[:, :, 1], bnst[:, :, 4]
        m2e, m2o = bnst[:, :, 2], bnst[:, :, 5]
        mu = pool.tile([128, nt], F32, tag=tagp + "_mu")
        rstd = pool.tile([128, nt], F32, tag=tagp + "_rstd")
        ssq = pool.tile([128, nt], F32, tag=tagp + "_ssq")
        dl = pool.tile([128, nt], F32, tag=tagp + "_dl")
        nc.vector.tensor_tensor(dl[:], me, mo, Alu.subtract)
        nc.vector.tensor_tensor(dl[:], dl[:], dl[:], Alu.mult)       # delta^2
        nc.vector.tensor_tensor(mu[:], me, mo, Alu.add)
        nc.vector.tensor_scalar_mul(mu[:], mu[:], 0.5)               # mean
        nc.vector.tensor_tensor(rstd[:], m2e, m2o, Alu.add)
        nc.vector.scalar_tensor_tensor(rstd[:], dl[:], float(DDIM) / 4.0, rstd[:],
                                       op0=Alu.mult, op1=Alu.add)    # M2 total
        nc.vector.tensor_tensor(ssq[:], mu[:], mu[:], Alu.mult)
        nc.vector.scalar_tensor_tensor(ssq[:], ssq[:], float(DDIM), rstd[:],
                                       op0=Alu.mult, op1=Alu.add)    # sum sq
        nc.vector.tensor_scalar(rstd[:], rstd[:], 1.0 / DDIM, eps,
                                op0=Alu.mult, op1=Alu.add)           # var + eps
        nc.scalar.activation(rstd[:], rstd[:], Act.Sqrt)
        nc.vector.reciprocal(rstd[:], rstd[:])
        return mu, rstd, ssq

    def feature_pass(src, nt, xnT_dst, h16_view, pool, tpool, psum, tagp,
                     psum2=None):
        psum2 = psum2 or psum
        """Processed in independent blocks of 8 tiles so block k+1's stats
        overlap block k's matmul/normalize sweeps (no global barrier)."""
        xall = pool.tile([128, nt, DDIM], F32, tag=tagp + "_xall")
        hall = pool.tile([128, nt, HDIM], F32, tag=tagp + "_hall")
        BLK = nt  # blocked variants measured slower (transient-tag serialization)
        for b0 in range(0, nt, BLK):
            xblk = xall[:, b0:b0 + BLK, :]
            hblk = hall[:, b0:b0 + BLK, :]
            for g0 in range(b0, b0 + BLK, 4):
                nc.sync.dma_start(
                    xall[:, g0:g0 + 4, :],
                    src.ap().rearrange("(g p) d -> p g d", p=128)[:, g0:g0 + 4, :])
            # LN stats for this block
            mu, rstd, _ = _moments(pool, BLK, xblk, f"{tagp}x{b0}", LN_EPS)
            # h = gelu(ln(x) @ w1); gelu batched 4-wide to amortize ACT overhead
            for g0 in range(b0, b0 + BLK, 4):
                ph4 = psum2.tile([128, 4, 128], F32, tag="fp_ph4")
                ptx4 = psum.tile([128, 4, 128], F32, tag="fp_ptx4")
                xlnT4 = tpool.tile([128, 4, DDIM], F32, tag="fp_xlnT4")
                for gi in range(4):
                    g = g0 + gi
                    xln = tpool.tile([128, DDIM], F32, tag="fp_xln")
                    nc.vector.tensor_scalar(xln[:], xall[:, g, :],
                                            mu[:, g - b0:g - b0 + 1],
                                            rstd[:, g - b0:g - b0 + 1],
                                            op0=Alu.subtract, op1=Alu.mult)
                    if cfg.lng:
                        nc.vector.tensor_tensor(xln[:], xln[:], LNG[:], Alu.mult)
                    if cfg.lnb:
                        nc.vector.tensor_tensor(xln[:], xln[:], LNB[:], Alu.add)
                    nc.tensor.transpose(ptx4[:, gi, :], xln[:], id32[:])
                nc.vector.tensor_copy(xlnT4[:], ptx4[:])
                for gi in range(4):
                    nc.tensor.matmul(ph4[:, gi, :], xlnT4[:, gi, :], w1s[:])
                    if cfg.b1:
                        nc.vector.tensor_tensor(ph4[:, gi, :], ph4[:, gi, :],
                                                B1R[:], Alu.add)
                nc.scalar.activation(hall[:, g0:g0 + 4, :], ph4[:], gelu_f)
            # h norms for this block
            _, _, ssqh = _moments(pool, BLK, hblk, f"{tagp}h{b0}", 0.0)
            invh = pool.tile([128, BLK], F32, tag=f"{tagp}_invh{b0}")
            nc.scalar.activation(invh[:], ssqh[:], Act.Sqrt)
            nc.vector.tensor_scalar_max(invh[:], invh[:], 1e-12)
            nc.vector.reciprocal(invh[:], invh[:])
            # h16 out (4-tile staging), xn = h/|h|, transpose to xnT_dst
            for g0 in range(b0, b0 + BLK, 4):
                h16s = tpool.tile([128, 4, HDIM], F16, tag="fp_h16s")
                nc.scalar.copy(h16s[:], hall[:, g0:g0 + 4, :])
                pt4 = psum.tile([128, 4, 128], F32, tag="fp_pt4")
                for gi in range(4):
                    g = g0 + gi
                    xn = tpool.tile([128, HDIM], F32, tag="fp_xn")
                    nc.vector.tensor_scalar_mul(xn[:], hall[:, g, :],
                                                invh[:, g - b0:g - b0 + 1])
                    nc.tensor.transpose(pt4[:, gi, :], xn[:], id32[:])
                nc.scalar.copy(xnT_dst[:, g0 * 128:(g0 + 4) * 128], pt4[:])
                nc.sync.dma_start(h16_view[:, g0:g0 + 4, :], h16s[:])

    h16v = h16_dram.ap().rearrange("p (g d) -> p g d", d=HDIM)
    with tc.tile_pool(name="p0", bufs=1) as p0w, \
         tc.tile_pool(name="p0t", bufs=4) as p0t, \
         tc.tile_pool(name="p0ps", bufs=3, space="PSUM") as p0ps, \
         tc.tile_pool(name="p0ps2", bufs=2, space="PSUM") as p0ps2:
        feature_pass(xf, NT, xnT[:], h16v, p0w, p0t, p0ps, "ff", psum2=p0ps2)
        for k in range(KCHEB):
            w2f = p0t.tile([128, ODIM], F32, tag="w2f")
            nc.sync.dma_start(w2f[:], w2e[k * 128:(k + 1) * 128, :])
            nc.vector.tensor_copy(w2s[:, k, :], w2f[:])
        # own-rows h: load now (ACT DGE queue), consumed by Gown/P2a in pass 2
        nc.scalar.dma_start(hm16[:],
                            h16_dram.ap()[:, 0:r].rearrange(
                                "p (t d) -> p t d", d=HDIM))

    # Node order is CORE-RELATIVE (host feeds xf rotated so this core's own
    # rows come first): own nodes = tiles [0, RT), partner = [RT, NT). Only
    # the gathers out of pair collectives need the runtime partner offset.
    pid = nc.partition_id()
    poff = ((pid + 1) % 2) * r if n > r else 0


    # =====================================================================
    # Pass 1: per row-tile: A block -> top-32 -> MT, deg
    # =====================================================================
    QW = 512 if n >= 512 else n
    NQ = n // QW
    with tc.tile_pool(name="p1", bufs=1) as p1, \
         tc.tile_pool(name="p1t", bufs=2) as p1t, \
         tc.tile_pool(name="p1psA", bufs=6, space="PSUM") as p1psA:
        # Two rotating A buffers + FIXED top-k scratches (cand/cext) + M16.
        # cand/cext are only ever touched by DVE (in program order), so no
        # cross-engine WAR between consecutive tiles; the mask of tile t and
        # the PE transposes run in the shadow of tile t+1's DVE work.
        SEG = cfg.seg
        NSEG = n // SEG if SEG else 0
        # Triple-buffered A: the relu evac of tile t+1 must not wait on the
        # (slow, Pool-resident) mask of tile t-1 still reading its buffer.
        Aq0 = p1.tile([128, n], F32, tag="Aq0")
        Aq1 = p1.tile([128, n], F32, tag="Aq1")
        Aq2 = p1.tile([128, n], F32, tag="Aq2")
        Aqs = (Aq0, Aq1, Aq2)
        Cfix = p1.tile([128, n if not SEG else 8 * NSEG], F32, tag="Cfix")
        if SEG:
            candF = p1.tile([128, 8 * NSEG], F32, tag="candF")
        else:
            candF = None
        M16a = p1.tile([128, n], F16, tag="M16a")
        M16b = p1.tile([128, n], F16, tag="M16b")

        def emit_A(t):
            """A row-block: fp16 matmuls (1 cyc/row) + relu into Aq[t%2].
            Own rows are node tiles [0, RT) of the core-relative order, so
            the lhsT is a static in-place slice of xnT."""
            A = Aqs[t % 3]
            xnm = xnT[:, t * 128:(t + 1) * 128]

            for q in range(NQ):
                pq = p1psA.tile([128, QW], F32, tag="pq")
                for s in range(QW // 512):
                    lo = s * 512
                    nc.tensor.matmul(pq[:, lo:lo + 512], xnm,
                                     xnT[:, q * QW + lo:q * QW + lo + 512])
                nc.scalar.activation(A[:, q * QW:(q + 1) * QW], pq[:], Act.Relu)

        def emit_rounds(t):
            """top-32 + degree for tile t; returns the top32 tile.

            Segmented: per-segment top-8 candidates (one full pass as NSEG
            cheap Max ops), then 4xMax8 + 3xMatchReplace over the small
            candidate tile. Exact unless a segment holds >8 of the row's true
            top-32 (measured: 26/16384 rows on the real data; each mildly
            perturbs one row)."""
            A = Aqs[t % 3]
            top8 = p1t.tile([128, 32], F32, tag="top8")
            if SEG:
                cand = candF
                for s in range(NSEG):
                    nc.vector.max(cand[:, s * 8:(s + 1) * 8],
                                  A[:, s * SEG:(s + 1) * SEG])
                src = cand
            else:
                src = A
            C = Cfix
            nc.vector.max(top8[:, 0:8], src[:])
            nc.vector.match_replace(C[:], top8[:, 0:8], src[:], -1.0)
            nc.vector.max(top8[:, 8:16], C[:])
            nc.vector.match_replace(C[:], top8[:, 8:16], C[:], -1.0)
            nc.vector.max(top8[:, 16:24], C[:])
            nc.vector.match_replace(C[:], top8[:, 16:24], C[:], -1.0)
            nc.vector.max(top8[:, 24:32], C[:])
            # degree = sum of the 32 extracted values (exact: the same 32 kept
            # values), accumulated on ACT to stay off the DVE round stream
            dsc = p1t.tile([128, 32], F32, tag="degscr")
            nc.scalar.activation(dsc[:], top8[:, 0:32], Act.Copy,
                                 accum_out=degM[:, t:t + 1])
            return top8

        def emit_mask(t, top8):
            """mask (gpsimd/DVE) + one whole-tile DMA transpose into MT."""
            A = Aqs[t % 3]
            M16 = M16a if t % 2 == 0 else M16b
            if cfg.mask_dve_mod and t % cfg.mask_dve_mod == 0:
                nc.vector.scalar_tensor_tensor(M16[:], A[:], top8[:, 31:32], A[:],
                                               op0=Alu.is_ge, op1=Alu.mult)
            else:
                nc.gpsimd.scalar_tensor_tensor(M16[:], A[:], top8[:, 31:32], A[:],
                                               op0=Alu.is_ge, op1=Alu.mult)
            nc.sync.dma_start_transpose(MT[:, t], M16[:])

        def emit_dm_chain():
            """deg -> dm12 vectors and the pair exchange; emitted between the
            last tile's rounds and its mask so the DMA/collective overlaps the
            remaining mask/transpose work."""
            DEG, DM, CDM1, Q1, Q2, CDM2 = range(6)
            nc.vector.tensor_scalar(dmv[:, DEG, :], degM[:], c1, c2,
                                    op0=Alu.mult, op1=Alu.add)
            nc.scalar.activation(dmv[:, DM, :], dmv[:, DEG, :], Act.Sqrt)
            nc.vector.reciprocal(dmv[:, DM, :], dmv[:, DM, :])
            nc.vector.tensor_scalar_mul(dmv[:, CDM1, :], dmv[:, DM, :], c1)
            nc.vector.tensor_tensor(dmv[:, Q1, :], dmv[:, DM, :], dmv[:, DM, :], Alu.mult)
            nc.vector.tensor_scalar_mul(dmv[:, Q2, :], dmv[:, Q1, :], 2.0 * c2)
            nc.vector.tensor_scalar_mul(dmv[:, Q1, :], dmv[:, Q1, :], c2)
            nc.vector.tensor_scalar_mul(dmv[:, CDM2, :], dmv[:, DM, :], 2.0 * c1)
            nc.sync.dma_start(dm_in.ap().rearrange("(t p) -> p t", p=128), dmv[:, DM, :])
            if cfg.use_cc:
                nc.gpsimd.collective_compute("AllGather", Alu.bypass,
                                             replica_groups=groups,
                                             ins=[dm_in[:].opt()], outs=[dm_out[:].opt()])
            else:
                nc.sync.dma_start(dm_out[0:r], dm_in[:])
                if n > r:
                    nc.sync.dma_start(dm_out[r:n], dm_in[:])
            nc.sync.dma_start(
                dm12oth[:],
                dm_out.ap()[bass.ds(poff, r)].rearrange("(g p) -> p g", p=128))

        # software-pipelined EMISSION: tile t+1's A-block goes into the PE/ACT
        # queues BEFORE tile t's mask/transposes, so the in-order engines never
        # park the next tile's prerequisites behind Pool-gated work.
        emit_A(0)
        for t in range(RT):
            if t + 1 < RT:
                emit_A(t + 1)
            top8 = emit_rounds(t)
            if t == RT - 1:
                emit_dm_chain()
            emit_mask(t, top8)

    early_stack.close()  # xnT dead after pass 1
    DEG, DM, CDM1, Q1, Q2, CDM2 = range(6)

    # =====================================================================
    # Pass 2/3: Chebyshev products, own-node half first (needs only LOCAL
    # degrees / local T1), partner half after each exchange; all transposes
    # ride the DMA xbar (dma_start_transpose) instead of PE+evac.
    # =====================================================================
    RG = max(r // 512, 1)
    RW = min(512, r)
    TPG = RW // 128
    OJS = list(range(RT))                          # own column tiles (static)
    XJS = list(range(RT, NT)) if n > r else []     # partner column tiles

    late = stack.enter_context(tc.tile_pool(name="late", bufs=1))
    xres_all = late.tile([128, RT, DDIM], F32, tag="xres_all")
    P2a = late.tile([128, RT, HDIM], F16, tag="P2a")
    Qta = late.tile([128, RT, HDIM], F16, tag="Qta")
    hTa = late.tile([128, RT, HDIM], F16, tag="hTa")
    T1loc = late.tile([128, RT, HDIM], F16, tag="T1loc")
    T2loc = late.tile([128, RT, HDIM], F16, tag="T2loc")
    T1T = late.tile([128, RT, HDIM], F16, tag="T1T")
    T2T = late.tile([128, RT, HDIM], F16, tag="T2T")
    Gown = late.tile([128, RT, HDIM], F16, tag="Gown")
    G2own = late.tile([128, RT, HDIM], F16, tag="G2own")
    if XJS:
        hf16o = late.tile([128, RT, HDIM], F16, tag="hf16o")
        Goth = late.tile([128, RT, HDIM], F16, tag="Goth")
        G2oth = Goth    # loaded from the exchange after Goth's last read

    if cfg.b2:
        B2R = late.tile([128, ODIM], F32, tag="B2R")
        nc.sync.dma_start(B2R[:], b2_e.ap().partition_broadcast(128))

    t1iv = t1_in.ap().rearrange("(t p) d -> p t d", p=128)
    h16vv = h16_dram.ap().rearrange("p (g d) -> p g d", d=HDIM)

    with tc.tile_pool(name="p2s", bufs=4) as p2s, \
         tc.tile_pool(name="p2ps", bufs=4, space="PSUM") as p2ps, \
         tc.tile_pool(name="pyps", bufs=2, space="PSUM") as pyps:
        # partner-h prefetch on the ACT DGE queue (hm16 was loaded back in
        # the feature pass); non-critical hTa/xres ride the SP queue.
        if XJS:
            nc.scalar.dma_start(hf16o[:], h16vv[:, RT:NT, :])
        nc.sync.dma_start_transpose(hTa[:], hm16[:])   # [H-part, t, row]
        nc.sync.dma_start(xres_all[:], xm.ap().rearrange("(t p) d -> p t d", p=128))

        # G (own): local dm12, no collective dependency
        for i in range(RT):
            nc.scalar.activation(Gown[:, i, :], hm16[:, i, :], Act.Copy,
                                 scale=dmv[:, DM, i:i + 1])
        for t in range(RT):
            nc.vector.tensor_scalar_mul(P2a[:, t, :], hm16[:, t, :],
                                        dmv[:, Q1, t:t + 1])

        # ---- T1: own-half accumulation for all strips ----
        pstrips = []
        for rg in range(RG):
            ps = p2ps.tile([128, RW], F32, tag="pstrip")
            for i, j in enumerate(OJS):
                nc.tensor.matmul(ps[:], Gown[:, i, :],
                                 MT[:, rg * TPG:(rg + 1) * TPG, j, :],
                                 start=(i == 0), stop=(not XJS and i == RT - 1))
            pstrips.append(ps)

        # G (partner): needs the gathered partner dm12
        if XJS:
            for i in range(RT):
                nc.scalar.activation(Goth[:, i, :], hf16o[:, i, :], Act.Copy,
                                     scale=dm12oth[:, i:i + 1])

        # ---- T1: partner half + combine + ship strips ----
        for rg in range(RG):
            ps = pstrips[rg]
            if XJS:
                for i, j in enumerate(XJS):
                    nc.tensor.matmul(ps[:], Goth[:, i, :],
                                     MT[:, rg * TPG:(rg + 1) * TPG, j, :],
                                     start=False, stop=(i == RT - 1))
            raw = p2s.tile([128, RW], F16, tag="raw16")
            nc.scalar.copy(raw[:], ps[:])
            t1rT = p2s.tile([128, TPG, HDIM], F16, tag="t1rT")
            nc.sync.dma_start_transpose(t1rT[:], raw[:])
            for ti in range(TPG):
                t = rg * TPG + ti
                nc.vector.scalar_tensor_tensor(T1loc[:, t, :], t1rT[:, ti, :],
                                               dmv[:, CDM1, t:t + 1],
                                               P2a[:, t, :],
                                               op0=Alu.mult, op1=Alu.add)
                # G2 = dm12*T1 is what every consumer wants — ship IT through
                # the exchange so the received bytes are directly usable
                nc.scalar.activation(G2own[:, t, :], T1loc[:, t, :], Act.Copy,
                                     scale=dmv[:, DM, t:t + 1])
            nc.sync.dma_start(t1iv[:, rg * TPG:(rg + 1) * TPG, :],
                              G2own[:, rg * TPG:(rg + 1) * TPG, :])
            nc.sync.dma_start_transpose(
                T1T[:, rg * TPG:(rg + 1) * TPG, :],
                T1loc[:, rg * TPG:(rg + 1) * TPG, :])

        # local prep while the G2 exchange is in flight
        for t in range(RT):
            nc.vector.scalar_tensor_tensor(Qta[:, t, :], T1loc[:, t, :],
                                           dmv[:, Q2, t:t + 1], hm16[:, t, :],
                                           op0=Alu.mult, op1=Alu.subtract)

        # G2 exchange (carries dm12-scaled T1 halves)
        if cfg.use_cc:
            nc.gpsimd.collective_compute("AllGather", Alu.bypass,
                                         replica_groups=groups,
                                         ins=[t1_in[:].opt()],
                                         outs=[t1_out[:].opt()])
        else:
            nc.sync.dma_start(t1_out[0:r, :], t1_in[:])
            if n > r:
                nc.sync.dma_start(t1_out[r:n, :], t1_in[:])

        # ---- T2: own-half accumulation ----
        pstrips2 = []
        for rg in range(RG):
            ps2 = p2ps.tile([128, RW], F32, tag="pstrip")
            for i, j in enumerate(OJS):
                nc.tensor.matmul(ps2[:], G2own[:, i, :],
                                 MT[:, rg * TPG:(rg + 1) * TPG, j, :],
                                 start=(i == 0), stop=(not XJS and i == RT - 1))
            pstrips2.append(ps2)

        if XJS:
            # partner half of the gathered G2 — ready to matmul as-is
            nc.sync.dma_start(
                G2oth[:],
                t1_out.ap()[bass.ds(poff, r), :].rearrange(
                    "(g p) d -> p g d", p=128))

        def out_tile(t):
            # y = [h,T1,T2] @ w2 (+b2);  out = x + tanh(gate)*y
            py = pyps.tile([128, ODIM], F32, tag="pyY")
            comps = (hTa, T1T, T2T)
            for k in range(KCHEB):
                nc.tensor.matmul(py[:], comps[k][:, t, :], w2s[:, k, :],
                                 start=(k == 0), stop=(k == KCHEB - 1))
            if cfg.b2:
                nc.vector.tensor_tensor(py[:], py[:], B2R[:], Alu.add)
            outt = p2s.tile([128, ODIM], F32, tag="outt")
            nc.vector.scalar_tensor_tensor(outt[:], py[:], tg, xres_all[:, t, :],
                                           op0=Alu.mult, op1=Alu.add)
            nc.scalar.dma_start(out_e[t * 128:(t + 1) * 128, :], outt[:])

        # ---- T2: partner half + combine + fused output stage ----
        for rg in range(RG):
            ps2 = pstrips2[rg]
            if XJS:
                for i, j in enumerate(XJS):
                    nc.tensor.matmul(ps2[:], G2oth[:, i, :],
                                     MT[:, rg * TPG:(rg + 1) * TPG, j, :],
                                     start=False, stop=(i == RT - 1))
            raw2 = p2s.tile([128, RW], F16, tag="raw16")
            nc.scalar.copy(raw2[:], ps2[:])
            t2rT = p2s.tile([128, TPG, HDIM], F16, tag="t2rT")
            nc.sync.dma_start_transpose(t2rT[:], raw2[:])
            for ti in range(TPG):
                t = rg * TPG + ti
                nc.vector.scalar_tensor_tensor(T2loc[:, t, :], t2rT[:, ti, :],
                                               dmv[:, CDM2, t:t + 1],
                                               Qta[:, t, :],
                                               op0=Alu.mult, op1=Alu.add)
            nc.sync.dma_start_transpose(
                T2T[:, rg * TPG:(rg + 1) * TPG, :],
                T2loc[:, rg * TPG:(rg + 1) * TPG, :])
            for ti in range(TPG):
                out_tile(rg * TPG + ti)

    stack.close()


def build(cfg, num_devices):
    nc = bacc.Bacc("TRN2", target_bir_lowering=False, debug=False,
                   num_devices=num_devices)
    with tile.TileContext(nc) as tc:
        _emit(nc, tc, cfg)
    nc.compile()
    return nc


def _host_scalars(log_tau, gate):
    tau = max(float(np.exp(np.float32(log_tau))), 1e-3)
    c1 = (1.0 - TELEPORT) / tau
    c2 = (1.0 - TELEPORT) / tau + TELEPORT
    tg = float(np.tanh(np.float32(gate)))
    return c1, c2, tg


def _flags(ln_g, ln_b, b1, b2):
    return (not np.all(ln_g == 1.0), not np.all(ln_b == 0.0),
            not np.all(b1 == 0.0), not np.all(b2 == 0.0))


_CACHE = {}


def kernel(x, ln_g, ln_b, w1, b1, w2, b2, log_tau, gate):
    x = np.ascontiguousarray(x, dtype=np.float32)
    assert x.shape == (BSZ, NFULL, DDIM), x.shape
    scalars = _host_scalars(log_tau, gate)
    flags = _flags(np.asarray(ln_g), np.asarray(ln_b), np.asarray(b1), np.asarray(b2))
    key = (scalars, flags)
    if key not in _CACHE:
        cfg = Cfg(NFULL, NFULL // 2, True, scalars, flags)
        _CACHE[key] = (build(cfg, N_CORES), cfg)
    nc, cfg = _CACHE[key]

    r = cfg.r
    base = {
        "w1e": np.ascontiguousarray(w1, np.float32),
        "w2e": np.ascontiguousarray(w2, np.float32),
        "lng": np.ascontiguousarray(ln_g, np.float32),
        "lnb": np.ascontiguousarray(ln_b, np.float32),
        "b1e": np.ascontiguousarray(b1, np.float32),
        "b2e": np.ascontiguousarray(b2, np.float32),
    }
    in_maps = []
    for c in range(N_CORES):
        b, j = c // 2, c % 2
        m = dict(base)
        own = x[b, j * r:(j + 1) * r, :]
        oth = x[b, (1 - j) * r:(2 - j) * r, :]
        # core-relative node order: own rows first (the kernel assumes it)
        m["xf"] = np.ascontiguousarray(np.concatenate([own, oth], axis=0))
        m["xm"] = np.ascontiguousarray(own)
        in_maps.append(m)

    res = run_bass_kernel_spmd(nc, in_maps, core_ids=list(range(N_CORES)))
    out = np.empty_like(x)
    for c in range(N_CORES):
        b, j = c // 2, c % 2
        out[b, j * r:(j + 1) * r, :] = res.results[c]["out"]
    return out

